# revision 4
# baseline (speedup 1.0000x reference)
"""AutoCorrelation layer kernel for 8 Trainium2 NeuronCores (v3, data-parallel).

Math note: the reference's rfft/irfft pair over the zero-padded head dim
computes a circular cross-correlation; its mean over all lags collapses
analytically to (sum_d q_proj) * (sum_d k_proj) per head:
  corr[b,l] = (1/(H*L)) * sum_h (q[b,l] @ WqS + bqS)_h * (k[b,l] @ WkS + bkS)_h
with WqS = Wq.reshape(D,H,DK).sum(-1).  Downstream (top-6, softmax, weighted
value gather, output projection) follows the reference directly.

Distribution: data-parallel preprocessing (core i handles batch i only:
corr row, top-6, softmax-weighted value aggregation -> agg[1,256]), an
AllGather of the tiny agg vector, then each core computes its own
32768-column shard of the (256, 262144) output projection.

Precision: the corr/top-k path is fp32 (top-k margins as small as 4e-4
relative make bf16/fp16 unsafe); Wp/v/Wv/agg/out are bf16 (4.2e-3 max rel
error, verified offline against the reference).  Top-6 membership is decided
by is_ge against the 6th-largest value; the compared values only pass
through bit-faithful copies and a x1.0 fp32 PE transpose whose few-ulp
perturbation is ~4 orders of magnitude below the smallest top-k gap.

The all-zero bias case (the reference's setup) compiles a variant that
skips bias loads/adds entirely; nonzero biases get the general variant.
"""
import sys

sys.path.insert(0, "/opt/trn_rl_repo")

import ml_dtypes
import numpy as np

import concourse.bass as bass
import concourse.mybir as mybir
import concourse.tile as tile
from concourse import bacc
from concourse.bass_utils import run_bass_kernel_spmd
from concourse.masks import make_identity

F32 = mybir.dt.float32
BF16 = mybir.dt.bfloat16
NPBF16 = ml_dtypes.bfloat16

N_CORES = 8
B, L, D, H, DK = 8, 1024, 256, 8, 32
K_TOP = 6
NSH = (L * D) // N_CORES          # 32768 output cols per core
TILE_N = 2048
N_TILES = NSH // TILE_N           # 16
SUBS = TILE_N // 512              # 4
SCALE = 1.0 / (H * L)
WP_BUFS = 14

TRACE = False          # test harness sets this for profiled runs
LAST_RESULT = None     # stashed BassKernelResults from the last kernel() call

_CACHE = {}


def _build_nc(with_bias):
    nc = bacc.Bacc("TRN2", target_bir_lowering=False, debug=False, num_devices=N_CORES)

    qt_d = nc.dram_tensor("qt", [128, 2 * L], F32, kind="ExternalInput").ap()
    kt_d = nc.dram_tensor("kt", [128, 2 * L], F32, kind="ExternalInput").ap()
    v_d = nc.dram_tensor("v", [128, 8 * D], BF16, kind="ExternalInput").ap()
    wq_d = nc.dram_tensor("wq", [128, 2 * D], F32, kind="ExternalInput").ap()
    wk_d = nc.dram_tensor("wk", [128, 2 * D], F32, kind="ExternalInput").ap()
    wv_d = nc.dram_tensor("wv", [128, 2 * D], BF16, kind="ExternalInput").ap()
    wp_d = nc.dram_tensor("wp", [128, 2 * NSH], BF16, kind="ExternalInput").ap()
    if with_bias:
        bq_d = nc.dram_tensor("bq", [1, D], F32, kind="ExternalInput").ap()
        bk_d = nc.dram_tensor("bk", [1, D], F32, kind="ExternalInput").ap()
        bv_d = nc.dram_tensor("bv", [1, D], F32, kind="ExternalInput").ap()
        bp_d = nc.dram_tensor("bp", [N_TILES, TILE_N], F32, kind="ExternalInput").ap()
    out_d = nc.dram_tensor("out", [B, NSH], BF16, kind="ExternalOutput").ap()

    with tile.TileContext(nc) as tc:
        with (
            tc.tile_pool(name="cst", bufs=1) as cst,
            tc.tile_pool(name="wpp", bufs=WP_BUFS) as wpp,
            tc.tile_pool(name="outp", bufs=3) as outp,
            tc.tile_pool(name="bpp", bufs=2) as bpp,
            tc.tile_pool(name="dr", bufs=1, space="DRAM") as dr,
            tc.tile_pool(name="ps_w", bufs=1, space="PSUM") as ps_w,
            tc.tile_pool(name="ps_r", bufs=1, space="PSUM") as ps_r,
            tc.tile_pool(name="ps_tp", bufs=2, space="PSUM") as ps_tp,
            tc.tile_pool(name="ps_out", bufs=3, space="PSUM") as ps_out,
        ):
            # ---------------- constants + PE warm-up ----------------
            one1 = cst.tile([1, 1], F32)
            nc.vector.memset(one1[:, :], 1.0)
            ones128 = cst.tile([128, 1], F32)
            nc.vector.memset(ones128[:, :], 1.0)
            sones = cst.tile([8, 1], F32)
            nc.vector.memset(sones[:, :], SCALE)
            ident8 = cst.tile([8, 8], F32)
            make_identity(nc, ident8[:, :])

            junk = cst.tile([128, 512], BF16)
            nc.vector.memset(junk[:, :], 0.01)
            wps = ps_w.tile([128, 512], F32, tag="warm")
            for i in range(5):
                nc.tensor.matmul(wps[:, :], junk[:, 0:128], junk[:, :],
                                 start=(i == 0), stop=(i == 4))
            junk2 = cst.tile([128, 512], F32)
            nc.vector.tensor_copy(junk2[:, :], wps[:, :])

            # ---------------- input DMAs ----------------
            # latency-critical loads lead the sync queue in dependency order;
            # the 16MB Wp shard streams behind them.  Bulk-but-late tensors
            # (v, wv, biases) ride the gpsimd queue.
            wq_sb = cst.tile([128, 2, D], F32)
            nc.sync.dma_start(wq_sb[:, :, :], wq_d.rearrange("p (c d) -> p c d", c=2))
            wk_sb = cst.tile([128, 2, D], F32)
            nc.sync.dma_start(wk_sb[:, :, :], wk_d.rearrange("p (c d) -> p c d", c=2))
            qt_sb = cst.tile([128, 2, L], F32)
            nc.sync.dma_start(qt_sb[:, :, :], qt_d.rearrange("p (c l) -> p c l", c=2))
            kt_sb = cst.tile([128, 2, L], F32)
            nc.sync.dma_start(kt_sb[:, :, :], kt_d.rearrange("p (c l) -> p c l", c=2))

            v_sb = cst.tile([128, 8, D], BF16)
            nc.gpsimd.dma_start(v_sb[:, :, :], v_d.rearrange("p (t d) -> p t d", t=8))
            wv_sb = cst.tile([128, 2, D], BF16)
            nc.gpsimd.dma_start(wv_sb[:, :, :], wv_d.rearrange("p (c d) -> p c d", c=2))
            if with_bias:
                bq_sb = cst.tile([1, D], F32)
                nc.gpsimd.dma_start(bq_sb[:, :], bq_d)
                bk_sb = cst.tile([1, D], F32)
                nc.gpsimd.dma_start(bk_sb[:, :], bk_d)
                bv_sb = cst.tile([1, D], F32)
                nc.gpsimd.dma_start(bv_sb[:, :], bv_d)
                bp_sb = cst.tile([N_TILES, TILE_N], F32)
                nc.gpsimd.dma_start(bp_sb[:, :], bp_d)

            # Wp shard: 16 x 1MB bf16 tiles streamed on the sync queue.
            wpts = []
            for nt in range(N_TILES):
                wpt = wpp.tile([128, 2, TILE_N], BF16, tag="wp")
                nc.sync.dma_start(
                    wpt[:, :, :],
                    wp_d[:, 2 * TILE_N * nt:2 * TILE_N * (nt + 1)]
                    .rearrange("p (c n) -> p c n", c=2))
                wpts.append(wpt)

            # ---------------- head sums of Wq/Wk ----------------
            wqs = cst.tile([128, 2, 8], F32)
            nc.vector.reduce_sum(out=wqs[:, :, :],
                                 in_=wq_sb[:, :, :].rearrange("p c (h z) -> p c h z", z=DK),
                                 axis=mybir.AxisListType.X)
            wks = cst.tile([128, 2, 8], F32)
            nc.vector.reduce_sum(out=wks[:, :, :],
                                 in_=wk_sb[:, :, :].rearrange("p c (h z) -> p c h z", z=DK),
                                 axis=mybir.AxisListType.X)
            if with_bias:
                bqs_row = cst.tile([1, 8], F32)
                nc.vector.reduce_sum(out=bqs_row[:, :],
                                     in_=bq_sb[:, :].rearrange("o (h z) -> o h z", z=DK),
                                     axis=mybir.AxisListType.X)
                bks_row = cst.tile([1, 8], F32)
                nc.vector.reduce_sum(out=bks_row[:, :],
                                     in_=bk_sb[:, :].rearrange("o (h z) -> o h z", z=DK),
                                     axis=mybir.AxisListType.X)

            # ---------------- q/k head-sum projections (fp32) ----------------
            # sq/sk land in PSUM; the scalar engine drains sq to SBUF (adding
            # the head bias in the general variant) while the PE moves on.
            ps_q = []
            ps_k = []
            for (t_sb, w_sum, ps_list) in ((qt_sb, wqs, ps_q), (kt_sb, wks, ps_k)):
                for half in range(2):
                    sl = slice(512 * half, 512 * (half + 1))
                    ps_x = ps_out.tile([8, 512], F32, tag="po")
                    nc.tensor.matmul(ps_x[:, :], w_sum[:, 0, :], t_sb[:, 0, sl], start=True, stop=False)
                    nc.tensor.matmul(ps_x[:, :], w_sum[:, 1, :], t_sb[:, 1, sl], start=False, stop=True)
                    ps_list.append(ps_x)

            if with_bias:
                bqs_ps = ps_tp.tile([8, 1], F32, tag="tp")
                nc.tensor.matmul(bqs_ps[:, :], bqs_row[:, :], one1[:, :], start=True, stop=True)
                bqs_vert = cst.tile([8, 1], F32)
                nc.vector.tensor_copy(bqs_vert[:, :], bqs_ps[:, :])
                bks_ps = ps_tp.tile([8, 1], F32, tag="tp")
                nc.tensor.matmul(bks_ps[:, :], bks_row[:, :], one1[:, :], start=True, stop=True)
                bks_vert = cst.tile([8, 1], F32)
                nc.vector.tensor_copy(bks_vert[:, :], bks_ps[:, :])

            sq_sb = cst.tile([8, L], F32)
            prod = cst.tile([8, L], F32)
            for half in range(2):
                sl = slice(512 * half, 512 * (half + 1))
                if with_bias:
                    nc.vector.tensor_scalar(
                        out=sq_sb[:, sl], in0=ps_q[half][:, :],
                        scalar1=bqs_vert[:, 0:1], scalar2=None,
                        op0=mybir.AluOpType.add)
                else:
                    nc.scalar.copy(sq_sb[:, sl], ps_q[half][:, :])
            for half in range(2):
                sl = slice(512 * half, 512 * (half + 1))
                if with_bias:
                    sk_sb = cst.tile([8, 512], F32, tag=f"sk{half}")
                    nc.vector.tensor_scalar(
                        out=sk_sb[:, :], in0=ps_k[half][:, :],
                        scalar1=bks_vert[:, 0:1], scalar2=None,
                        op0=mybir.AluOpType.add)
                    nc.vector.tensor_mul(prod[:, sl], sq_sb[:, sl], sk_sb[:, :])
                else:
                    # DVE reads the sk chunk straight from PSUM
                    nc.vector.tensor_mul(prod[:, sl], sq_sb[:, sl], ps_k[half][:, :])

            ps_rr = ps_r.tile([1, L], F32, tag="r")
            for half in range(2):
                sl = slice(512 * half, 512 * (half + 1))
                nc.tensor.matmul(ps_rr[:, sl], sones[:, :], prod[:, sl], start=True, stop=True)
            r_row = cst.tile([1, L], F32)
            nc.scalar.copy(r_row[:, :], ps_rr[:, :])

            # ---------------- top-6 + masked softmax weights ----------------
            topv = cst.tile([1, 8], F32)
            nc.vector.max(topv[:, :], r_row[:, :])
            tvb = cst.tile([128, 8], F32)
            nc.gpsimd.partition_broadcast(tvb[:, :], topv[:, :])
            ntv0 = cst.tile([128, 1], F32)
            nc.vector.tensor_scalar_mul(ntv0[:, :], tvb[:, 0:1], -1.0)

            # rT[p, c] = r[128 c + p] via K=1 transpose-matmuls
            ps_rT = ps_tp.tile([128, 8], F32, tag="tp")
            for c in range(8):
                nc.tensor.matmul(ps_rT[:, c:c + 1], r_row[0:1, 128 * c:128 * (c + 1)],
                                 one1[:, :], start=True, stop=True)
            rT = cst.tile([128, 8], F32)
            nc.vector.tensor_copy(rT[:, :], ps_rT[:, :])

            eT = cst.tile([128, 8], F32)
            nc.scalar.activation(eT[:, :], rT[:, :],
                                 mybir.ActivationFunctionType.Exp,
                                 bias=ntv0[:, 0:1], scale=1.0)
            maskT = cst.tile([128, 8], F32)
            nc.vector.tensor_scalar(
                out=maskT[:, :], in0=rT[:, :],
                scalar1=tvb[:, K_TOP - 1:K_TOP], scalar2=None,
                op0=mybir.AluOpType.is_ge)
            ze = cst.tile([128, 8], F32)
            nc.vector.tensor_mul(ze[:, :], eT[:, :], maskT[:, :])
            ze_bf = cst.tile([128, 8], BF16)
            nc.vector.tensor_copy(ze_bf[:, :], ze[:, :])
            s1 = cst.tile([128, 1], F32)
            nc.vector.reduce_sum(out=s1[:, :], in_=ze[:, :], axis=mybir.AxisListType.X)
            Zps = ps_tp.tile([1, 1], F32, tag="tp")
            nc.tensor.matmul(Zps[:, :], s1[:, 0:1], ones128[:, :], start=True, stop=True)
            Zsb = cst.tile([1, 1], F32)
            nc.vector.tensor_copy(Zsb[:, :], Zps[:, :])
            zinv = cst.tile([1, 1], F32)
            nc.vector.reciprocal(zinv[:, :], Zsb[:, :])
            zvb = cst.tile([128, 1], F32)
            nc.gpsimd.partition_broadcast(zvb[:, :], zinv[:, :])

            # ---------------- weighted value aggregation (PE) ----------------
            # vbar[e] = (1/Z) sum_l v[l, e] * ze[l]; softmax normalization is
            # folded into the PSUM drain so the PE never waits on zvb.
            vbarT = cst.tile([128, 2], BF16)
            for m in range(2):
                pv = ps_tp.tile([128, 1], F32, tag="tp")
                for t in range(8):
                    nc.tensor.matmul(pv[:, :], v_sb[:, t, 128 * m:128 * (m + 1)],
                                     ze_bf[:, t:t + 1], start=(t == 0), stop=(t == 7))
                nc.vector.tensor_scalar_mul(vbarT[:, m:m + 1], pv[:, :], zvb[:, 0:1])

            # agg[d'] = sum_e Wv[e, d'] vbar[e] (+ bv)  -> [128, 2] (d' halves)
            aggT2 = cst.tile([128, 2], F32)
            for m in range(2):
                pa = ps_tp.tile([128, 1], F32, tag="tp")
                nc.tensor.matmul(pa[:, :], wv_sb[:, 0, 128 * m:128 * (m + 1)],
                                 vbarT[:, 0:1], start=True, stop=(False if with_bias else False))
                nc.tensor.matmul(pa[:, :], wv_sb[:, 1, 128 * m:128 * (m + 1)],
                                 vbarT[:, 1:2], start=False, stop=not with_bias)
                if with_bias:
                    nc.tensor.matmul(pa[:, :], bv_sb[0:1, 128 * m:128 * (m + 1)],
                                     one1[:, :], start=False, stop=True)
                nc.vector.tensor_copy(aggT2[:, m:m + 1], pa[:, :])

            # ---------------- AllGather agg -> [8, 256] ----------------
            agg_in = dr.tile([1, D], F32)
            nc.gpsimd.dma_start(
                agg_in[:, :].rearrange("o (m e) -> (o e) m", e=128), aggT2[:, :])
            agg_out = dr.tile([B, D], F32)
            nc.gpsimd.collective_compute(
                "AllGather", mybir.AluOpType.bypass,
                replica_groups=[list(range(N_CORES))],
                ins=[agg_in[:, :].opt()], outs=[agg_out[:, :].opt()])
            aggf = cst.tile([8, D], F32)
            nc.gpsimd.dma_start(aggf[:, :], agg_out[:, :])
            aggt_bf = cst.tile([128, 16], BF16)
            for m in range(2):
                pt = ps_tp.tile([128, 8], F32, tag="tp")
                nc.tensor.transpose(pt[:, :], aggf[0:8, 128 * m:128 * (m + 1)], ident8[:, :])
                nc.vector.tensor_copy(aggt_bf[:, 8 * m:8 * (m + 1)], pt[:, :])

            # ---------------- big output projection (column shard) ----------------
            for nt in range(N_TILES):
                wpt = wpts[nt]
                if with_bias:
                    bp_rep = bpp.tile([8, TILE_N], F32, tag="bprep")
                    nc.gpsimd.partition_broadcast(bp_rep[:, :], bp_sb[nt:nt + 1, :])
                o_sb = outp.tile([8, TILE_N], BF16)
                for s in range(SUBS):
                    ssl = slice(512 * s, 512 * (s + 1))
                    ps = ps_out.tile([8, 512], F32, tag="po")
                    nc.tensor.matmul(ps[:, :], aggt_bf[:, 0:8], wpt[:, 0, ssl], start=True, stop=False)
                    nc.tensor.matmul(ps[:, :], aggt_bf[:, 8:16], wpt[:, 1, ssl], start=False, stop=True)
                    if s % 2 == 0:
                        nc.scalar.copy(o_sb[:, ssl], ps[:, :])
                    else:
                        nc.vector.tensor_copy(o_sb[:, ssl], ps[:, :])
                if with_bias:
                    nc.vector.tensor_add(o_sb[:, :], o_sb[:, :], bp_rep[:, :])
                nc.scalar.dma_start(out_d[:, TILE_N * nt:TILE_N * (nt + 1)], o_sb[:, :])

    nc.finalize()
    return nc


def _get_nc(with_bias):
    key = ("dp", with_bias)
    if key not in _CACHE:
        _CACHE[key] = _build_nc(with_bias)
    return _CACHE[key]


def kernel(queries, keys, values, Wq, bq, Wk, bk, Wv, bv, Wp, bp):
    q3 = np.asarray(queries, np.float32).reshape(B, L, D)
    k3 = np.asarray(keys, np.float32).reshape(B, L, D)
    v3 = np.asarray(values, np.float32).reshape(B, L, D)
    Wq = np.asarray(Wq, np.float32)
    Wk = np.asarray(Wk, np.float32)
    Wv = np.asarray(Wv, np.float32)
    bq = np.asarray(bq, np.float32).reshape(1, D)
    bk = np.asarray(bk, np.float32).reshape(1, D)
    bv = np.asarray(bv, np.float32).reshape(1, D)
    Wp = np.asarray(Wp, np.float32)
    bp = np.asarray(bp, np.float32).reshape(-1)

    with_bias = bool(bq.any() or bk.any() or bv.any() or bp.any())
    nc = _get_nc(with_bias)

    # shared (replicated) weight layouts
    wq_h = np.ascontiguousarray(
        Wq.reshape(2, 128, D).transpose(1, 0, 2).reshape(128, 2 * D))
    wk_h = np.ascontiguousarray(
        Wk.reshape(2, 128, D).transpose(1, 0, 2).reshape(128, 2 * D))
    wv_h = np.ascontiguousarray(
        Wv.reshape(2, 128, D).transpose(1, 0, 2).reshape(128, 2 * D)).astype(NPBF16)

    wp_bf = Wp.astype(NPBF16)

    in_maps = []
    for i in range(N_CORES):
        cols = slice(NSH * i, NSH * (i + 1))
        # per-batch transposed q/k: [p, c, l] with d = 128 c + p
        qt_h = np.ascontiguousarray(
            q3[i].T.reshape(2, 128, L).transpose(1, 0, 2).reshape(128, 2 * L))
        kt_h = np.ascontiguousarray(
            k3[i].T.reshape(2, 128, L).transpose(1, 0, 2).reshape(128, 2 * L))
        # v in [p, t, d] with l = 128 t + p
        v_h = np.ascontiguousarray(
            v3[i].reshape(8, 128, D).transpose(1, 0, 2).reshape(128, 8 * D)).astype(NPBF16)
        # Wp shard in [p, nt, c, j] with d = 128 c + p, col = 2048 nt + j
        wp_h = np.ascontiguousarray(
            wp_bf[:, cols].reshape(2, 128, N_TILES, TILE_N)
            .transpose(1, 2, 0, 3).reshape(128, 2 * NSH))
        m = {
            "qt": qt_h, "kt": kt_h, "v": v_h,
            "wq": wq_h, "wk": wk_h, "wv": wv_h,
            "wp": wp_h,
        }
        if with_bias:
            m["bq"] = bq
            m["bk"] = bk
            m["bv"] = bv
            m["bp"] = np.ascontiguousarray(bp[cols]).reshape(N_TILES, TILE_N)
        in_maps.append(m)

    res = run_bass_kernel_spmd(nc, in_maps, core_ids=list(range(N_CORES)), trace=TRACE)
    global LAST_RESULT
    LAST_RESULT = res
    out = np.concatenate(
        [np.asarray(res.results[i]["out"]) for i in range(N_CORES)], axis=1)
    return out.astype(np.float32).reshape(B, L, D)
